# revision 18
# baseline (speedup 1.0000x reference)
"""Causal multi-head attention (B=1, S=4096, D=1024, H=16, HD=64) on 8 TRN2
NeuronCores.

Sharding: tensor-parallel over heads. Core c owns heads [2c, 2c+1]: Wq/Wk/Wv
column slices (128 cols) and Wo row slice (128 rows). Each core computes its
partial output projection over the full sequence (bf16 partials, contiguous
unit-major layout); the host sums the 8 partials and adds bo.

Device kernel (per core; bf16 matmul operands, fp32 PSUM):
  - x fed pre-transposed (xT [D, S]); QT/KT = W.T @ xT, no on-chip transpose.
  - scores computed TRANSPOSED per 128-row sk tile: scT[sk, sq] = K_h @ Q_h^T
    (the two heads' 64-contraction matmuls run concurrently via PE row
    groups); 1/sqrt(HD) folded into Wq/bq on the host.
  - softmax: table EXP on ACT into bf16; causal narrowing (diagonal sk tiles
    only compute/exp/accumulate sq in [128t, 512)); one DVE triu-mask
    multiply on the 128-wide diagonal band.
  - denominators from a ones-column appended to V (row 64 of the attn@V
    accumulator); reciprocal (DVE, straight from PSUM), partition-broadcast
    (gpsimd), rescale attn-out^T before the output projection.
  - all heavy PE work that is off the attention critical path (QKV
    projections for chunk c+1, V transposes, output projection for chunk
    c-1) is split into small units and drained as PE fillers at the TOP of
    each attention iteration, so the in-order PE queue never blocks behind
    a semaphore-waiting scores matmul and the PE stays continuously busy.
  - last chunk's rescale + output projection run in 128-column pieces to
    pipeline the tail (proj copies there go on the then-idle ACT).
"""
import sys

sys.path.insert(0, "/opt/trn_rl_repo")

import numpy as np

import concourse.bacc as bacc
import concourse.mybir as mybir
import concourse.tile as tile
from concourse.bass_utils import run_bass_kernel_spmd
from concourse.dve_ops import RECIPROCAL_APPROX_FAST, RECIP_APPROX_FAST_CONSTS

F32 = mybir.dt.float32
BF16 = mybir.dt.bfloat16
I16 = mybir.dt.int16
COPYF = mybir.ActivationFunctionType.Copy
EXPF = mybir.ActivationFunctionType.Exp

# Schraudolph bf16-bits exp for the DVE-offloaded tiles:
# int16 bits = round(x * SCH_A + SCH_B) ~= bf16(exp(x))
SCH_A = float(128.0 / np.log(2.0))
SCH_B = 127.0 * 128.0 - 5.5

S, D, H, HD = 4096, 1024, 16, 64
NCORES = 8
CPC = D // NCORES  # 128 head-dim columns per core (2 heads)
NS = S // 512      # 8 chunks of 512 along the sequence
ND = D // 128      # 8 contraction chunks for the projections


def _build_nc():
    nc = bacc.Bacc("TRN2", target_bir_lowering=False, debug=False,
                   num_devices=NCORES)
    xT = nc.dram_tensor("xT", [D, S], BF16, kind="ExternalInput").ap()
    w3 = nc.dram_tensor("w3", [128, 3, ND, 128], BF16, kind="ExternalInput").ap()
    wo = nc.dram_tensor("wo", [128, D], BF16, kind="ExternalInput").ap()
    b3 = nc.dram_tensor("b3", [CPC, 3], F32, kind="ExternalInput").ap()
    tri = nc.dram_tensor("tri", [128, 2, 128], BF16, kind="ExternalInput").ap()
    # batched unit-major output: out[c, half, p, u, n, f] =
    #   partial[128*(4c + 2*half + u) + p, 512n + f]
    out = nc.dram_tensor("out", [NS, 2, 128, 2, 2, 512], BF16,
                         kind="ExternalOutput").ap()

    with tile.TileContext(nc) as tc:
        _emit(nc, tc, xT, w3, wo, b3, tri, out)
    nc.compile()
    return nc


def _emit(nc, tc, xT, w3, wo, b3, tri, out):
    from collections import deque
    from contextlib import ExitStack
    ctx = ExitStack()
    with ctx:
        consts = ctx.enter_context(tc.tile_pool(name="consts", bufs=1))
        persist = ctx.enter_context(tc.tile_pool(name="persist", bufs=1))
        xt_pool = ctx.enter_context(tc.tile_pool(name="xt", bufs=3))
        vt_pool = ctx.enter_context(tc.tile_pool(name="vt", bufs=2))
        pt_pool = ctx.enter_context(tc.tile_pool(name="pt", bufs=8))
        pti_pool = ctx.enter_context(tc.tile_pool(name="pti", bufs=4))
        rec_pool = ctx.enter_context(tc.tile_pool(name="rec", bufs=8))
        bc_pool = ctx.enter_context(tc.tile_pool(name="bc", bufs=4))
        ost_pool = ctx.enter_context(tc.tile_pool(name="ost", bufs=3))
        ps_mm = ctx.enter_context(tc.tile_pool(name="psmm", bufs=2, space="PSUM"))
        ps_sc = ctx.enter_context(tc.tile_pool(name="pssc", bufs=2, space="PSUM"))
        ps_ot = ctx.enter_context(tc.tile_pool(name="psot", bufs=2, space="PSUM"))

        # ---- constants (wq+wk slices first: the preamble needs them) -----
        w_sb = consts.tile([128, 3, ND, 128], BF16)
        nc.sync.dma_start(out=w_sb[:, 0:2], in_=w3[:, 0:2])
        b3_sb = consts.tile([CPC, 3], F32)
        bq_sb, bk_sb, bv_sb = b3_sb[:, 0:1], b3_sb[:, 1:2], b3_sb[:, 2:3]
        ti_sb = consts.tile([128, 2, 128], BF16)
        tri_sb, id_sb = ti_sb[:, 0], ti_sb[:, 1]
        wo_sb = consts.tile([128, D], BF16)

        def emit_small_consts():
            nc.sync.dma_start(out=b3_sb, in_=b3)
            nc.sync.dma_start(out=ti_sb, in_=tri)

        # ---- persistent activations -----------------------------------
        QT = persist.tile([128, S], BF16)     # [c(2 heads x 64), s]
        KT = persist.tile([128, S], BF16)
        VP = persist.tile([128, 2, 32, 65], BF16)  # V natural + ones col
        OT = persist.tile([128, S], BF16)     # rescaled attn out^T, 2 heads

        xT_k = xT.rearrange("(k p) s -> p k s", p=128)

        def load_xt(c):
            xt = xt_pool.tile([128, ND, 512], BF16, tag="xt")
            cc0 = slice(512 * c, 512 * (c + 1))
            nc.sync.dma_start(out=xt[:, 0:4], in_=xT_k[:, 0:4, cc0])
            nc.sync.dma_start(out=xt[:, 4:ND], in_=xT_k[:, 4:ND, cc0])
            return [xt[:, k] for k in range(ND)]

        # ---- filler machinery -----------------------------------------
        qkv_q = deque()   # QKV/transpose units (deadline: end of prev loop)
        proj_q = deque()  # output-projection units (soft deadline)

        def mk_proj_unit(i, half, st, cc, which):
            # one half (4 accumulation steps) of projection `which` in
            # (q, k, v); the second half adds the bias
            def u():
                if half == 0:
                    st[i] = ps_mm.tile([128, 512], F32, tag="mm",
                                       name=f"pqkv{i}")
                p_ps = st[i]
                for k in range(4 * half, 4 * half + 4):
                    nc.tensor.matmul(p_ps, w_sb[:, i, k], st["xts"][k],
                                     start=(k == 0), stop=(k == ND - 1))
                if half == 1:
                    if i == 0:
                        nc.vector.tensor_scalar_add(QT[:, cc], p_ps, bq_sb)
                    elif i == 1:
                        nc.vector.tensor_scalar_add(KT[:, cc], p_ps, bk_sb)
                    else:
                        st["vt"] = vt_pool.tile([128, 512], BF16, tag="vt",
                                                name="vt")
                        nc.vector.tensor_scalar_add(st["vt"], p_ps, bv_sb)
            return u

        def mk_tr_unit(c, t, st):
            def u():
                j = 4 * c + t
                tr_ps = ps_mm.tile([128, 128], BF16, tag="mm", name="trps")
                nc.tensor.transpose(tr_ps, st["vt"][:, 128 * t:128 * (t + 1)],
                                    id_sb)
                nc.vector.tensor_copy(VP[:, 0, j, 0:64], tr_ps[:, 0:64])
                nc.vector.tensor_copy(VP[:, 1, j, 0:64], tr_ps[:, 64:128])
                nc.vector.memset(VP[:, 0, j, 64:65], 1.0)
                nc.vector.memset(VP[:, 1, j, 64:65], 1.0)
            return u

        def enqueue_qkv(c, xts, skip_qk=False):
            cc = slice(512 * c, 512 * (c + 1))
            st = {"xts": xts}
            for i in range(3):
                if skip_qk and i < 2:
                    continue
                for half in (0, 1):
                    qkv_q.append(mk_proj_unit(i, half, st, cc, i))
            for t in range(4):
                qkv_q.append(mk_tr_unit(c, t, st))
            return st

        ogrp = {}  # (c, half) -> [staging tile, units done]

        def emit_out_unit(c, t, n, copy_on_act=False, split_dma=False):
            key = (c, t // 2)
            if key not in ogrp:
                ogrp[key] = [ost_pool.tile([128, 2, 2, 512], BF16, tag="ost",
                                           name="obig"), 0]
            o_big = ogrp[key][0]
            ss = slice(128 * (4 * c + t), 128 * (4 * c + t + 1))
            nn = slice(512 * n, 512 * (n + 1))
            pr_ps = ps_mm.tile([128, 512], F32, tag="mm", name="prps")
            nc.tensor.matmul(pr_ps, OT[:, ss], wo_sb[:, nn],
                             start=True, stop=True)
            if copy_on_act:
                nc.scalar.activation(out=o_big[:, t % 2, n], in_=pr_ps,
                                     func=COPYF)
            else:
                nc.vector.tensor_copy(o_big[:, t % 2, n], pr_ps)
            ogrp[key][1] += 1
            if split_dma and ogrp[key][1] % 2 == 0:
                nc.sync.dma_start(out=out[c, t // 2][:, t % 2],
                                  in_=o_big[:, t % 2])
                if ogrp[key][1] == 4:
                    del ogrp[key]
            elif not split_dma and ogrp[key][1] == 4:
                nc.sync.dma_start(out=out[c, t // 2], in_=o_big)
                del ogrp[key]

        def drain_one():
            if qkv_q:
                qkv_q.popleft()()
                return True
            if proj_q:
                emit_out_unit(*proj_q.popleft())
                return True
            return False

        # ---- preamble --------------------------------------------------
        # PE p-state warmup on zeros while the first DMAs land
        warm = consts.tile([128, 512], BF16)
        nc.vector.memset(warm, 0.0)
        wps = ps_mm.tile([128, 512], F32, tag="mm", name="wps")
        for _ in range(8):
            nc.tensor.matmul(wps, warm[:, 0:128], warm, start=True, stop=True)
        xts0 = load_xt(0)
        emit_small_consts()
        xts1 = load_xt(1)
        nc.sync.dma_start(out=w_sb[:, 2], in_=w3[:, 2])
        nc.sync.dma_start(out=wo_sb, in_=wo)
        st0 = {"xts": xts0}
        for i in (0, 1):          # chunk-0 Q and K projections, inline
            for half in (0, 1):
                mk_proj_unit(i, half, st0, slice(0, 512), i)()
        # chunk-0 V + transposes and chunk-1 QKV go through the filler queue
        st0b = enqueue_qkv(0, xts0, skip_qk=True)

        xts_next = xts1
        for c in range(NS):
            cc = slice(512 * c, 512 * (c + 1))
            if 0 < c < NS - 1:
                enqueue_qkv(c + 1, xts_next)
                xts_next = load_xt(c + 2) if c + 2 < NS else None

            njt = 4 * (c + 1)
            ot0 = ps_ot.tile([65, 512], F32, tag="ot", name="ot0")
            ot1 = ps_ot.tile([65, 512], F32, tag="ot", name="ot1")

            def emit_av(j, lo, pt, njt=njt, ot0=ot0, ot1=ot1):
                st, sp = (j == 0), (j == njt - 1)
                nc.tensor.matmul(ot0[:, lo:512], VP[:, 0, j], pt[:, 0, lo:512],
                                 start=st, stop=sp, skip_group_check=True)
                nc.tensor.matmul(ot1[:, lo:512], VP[:, 1, j], pt[:, 1, lo:512],
                                 start=st, stop=sp, skip_group_check=True)

            # how many fillers to drain per iteration to empty qkv_q in time
            pending = []
            for j in range(njt):
                need = len(qkv_q) + max(0, len(proj_q) - 4)
                rate = max(1, -(-need // max(1, njt - j)))
                for _ in range(min(rate, 3 if njt <= 8 else 8)):
                    drain_one()
                jj = slice(128 * j, 128 * (j + 1))
                t = j - 4 * c
                lo = 128 * t if t >= 0 else 0
                cl = slice(512 * c + lo, 512 * (c + 1))
                sc = ps_sc.tile([128, 2, 512], F32, tag="sc", name="sc")
                nc.tensor.matmul(sc[:, 0, lo:512], KT[0:64, jj], QT[0:64, cl],
                                 start=True, stop=True)
                nc.tensor.matmul(sc[:, 1, lo:512], KT[64:128, jj],
                                 QT[64:128, cl], start=True, stop=True)
                if c >= 2 and t < 0 and j % 4 == 2:
                    # offload this tile's exp to the DVE (Schraudolph
                    # affine into bf16 bit patterns)
                    pti = pti_pool.tile([128, 2, 512], I16, tag="pti",
                                        name="pti")
                    nc.vector.tensor_scalar(pti, sc[:, :, :],
                                            SCH_A, SCH_B,
                                            mybir.AluOpType.mult,
                                            mybir.AluOpType.add)
                    pt = pti.bitcast(BF16)
                else:
                    pt = pt_pool.tile([128, 2, 512], BF16, tag="pt", name="pt")
                    nc.scalar.activation(out=pt[:, :, lo:512],
                                         in_=sc[:, :, lo:512], func=EXPF)
                    if t >= 0:  # diagonal 128-wide band: triu mask
                        for h in (0, 1):
                            nc.vector.tensor_mul(pt[:, h, lo:lo + 128],
                                                 pt[:, h, lo:lo + 128], tri_sb)
                pending.append((j, lo, pt))
                if len(pending) > 2:
                    emit_av(*pending.pop(0))
            # flush: interleave fillers between the exp-blocked tail avs
            for p in pending:
                drain_one()
                if len(qkv_q) + len(proj_q) >= 6:
                    drain_one()
                emit_av(*p)

            if c < NS - 1:
                # softmax denominators (row 64) -> rescale attn-out^T
                bcs = []
                for h, ot in ((0, ot0), (1, ot1)):
                    den = rec_pool.tile([1, 512], F32, tag="den", name="den")
                    nc.vector.tensor_copy(den, ot[64:65, :])
                    rec = rec_pool.tile([1, 512], F32, tag="rec", name="rec")
                    nc.vector._custom_dve(RECIPROCAL_APPROX_FAST, out=rec,
                                          in0=den,
                                          s0=RECIP_APPROX_FAST_CONSTS["s0"],
                                          s1=RECIP_APPROX_FAST_CONSTS["s1"],
                                          imm2=RECIP_APPROX_FAST_CONSTS["imm2"])
                    bc = bc_pool.tile([64, 512], F32, tag="bc", name="bc")
                    nc.gpsimd.partition_broadcast(bc, rec)
                    bcs.append(bc)
                for h, ot in ((0, ot0), (1, ot1)):
                    nc.vector.tensor_mul(OT[64 * h:64 * (h + 1), cc],
                                         ot[0:64, :], bcs[h])
                if c == 0:
                    enqueue_qkv(1, xts_next)
                    xts_next = load_xt(2)
                while qkv_q:       # chunk c+1's QKV must be done before its loop
                    drain_one()
                proj_q.extend((c, t, n, False) for t in range(4)
                              for n in range(2))
            else:
                # tail: 128-col pieces pipelined (rescale_t || proj_{t-1});
                # copies on the now-idle ACT
                while qkv_q:
                    drain_one()
                for t in range(4):
                    pp = slice(128 * t, 128 * (t + 1))
                    ccp = slice(512 * c + 128 * t, 512 * c + 128 * (t + 1))
                    for h, ot in ((0, ot0), (1, ot1)):
                        den = rec_pool.tile([1, 128], F32, tag="den",
                                            name="den")
                        nc.vector.tensor_copy(den, ot[64:65, pp])
                        rec = rec_pool.tile([1, 128], F32, tag="rec",
                                            name="rec")
                        nc.vector._custom_dve(
                            RECIPROCAL_APPROX_FAST, out=rec, in0=den,
                            s0=RECIP_APPROX_FAST_CONSTS["s0"],
                            s1=RECIP_APPROX_FAST_CONSTS["s1"],
                            imm2=RECIP_APPROX_FAST_CONSTS["imm2"])
                        bc = bc_pool.tile([64, 128], F32, tag="bc", name="bc")
                        nc.gpsimd.partition_broadcast(bc, rec)
                        nc.vector.tensor_mul(OT[64 * h:64 * (h + 1), ccp],
                                             ot[0:64, pp], bc)
                    if proj_q:
                        emit_out_unit(*proj_q.popleft())
                    if t > 0:
                        emit_out_unit(c, t - 1, 0, True, split_dma=True)
                        emit_out_unit(c, t - 1, 1, True, split_dma=True)
                emit_out_unit(c, 3, 0, True, split_dma=True)
                emit_out_unit(c, 3, 1, True, split_dma=True)

        while proj_q:
            emit_out_unit(*proj_q.popleft())


_NC_CACHE = {}


def _get_nc():
    if "nc" not in _NC_CACHE:
        _NC_CACHE["nc"] = _build_nc()
    return _NC_CACHE["nc"]


def make_in_maps(x, Wq, bq, Wk, bk, Wv, bv, Wo, bo):
    import ml_dtypes
    cdt = ml_dtypes.bfloat16
    x = np.asarray(x, np.float32).reshape(S, D)
    xT = np.ascontiguousarray(x.T).astype(cdt)
    scale = 1.0 / np.sqrt(HD)
    tri = np.stack([np.triu(np.ones((128, 128), np.float32)),
                    np.eye(128, dtype=np.float32)], axis=1)
    in_maps = []
    for c in range(NCORES):
        cs = slice(CPC * c, CPC * (c + 1))
        # w3[p, proj, k, c2] = W[128k+p, c2] for the three projections
        w3 = np.stack([np.asarray(Wq)[:, cs] * scale,
                       np.asarray(Wk)[:, cs],
                       np.asarray(Wv)[:, cs]], axis=1)  # [D, 3, 128]
        w3 = np.ascontiguousarray(
            w3.reshape(ND, 128, 3, CPC).transpose(1, 2, 0, 3)).astype(cdt)
        in_maps.append({
            "xT": xT,
            "w3": w3,
            "wo": np.ascontiguousarray(np.asarray(Wo)[cs, :]).astype(cdt),
            "b3": np.stack([np.asarray(bq)[cs] * scale,
                            np.asarray(bk)[cs],
                            np.asarray(bv)[cs]], axis=1).astype(np.float32),
            "tri": tri.astype(cdt),
        })
    return in_maps


def kernel(x, Wq, bq, Wk, bk, Wv, bv, Wo, bo, _run_kwargs=None):
    nc = _get_nc()
    in_maps = make_in_maps(x, Wq, bq, Wk, bk, Wv, bv, Wo, bo)
    res = run_bass_kernel_spmd(nc, in_maps, list(range(NCORES)),
                               **(_run_kwargs or {}))
    acc = np.zeros((NS, 2, 128, 2, 2, 512), np.float32)
    for c in range(NCORES):
        acc += res.results[c]["out"].astype(np.float32)
    # (c, half, p, u, n, f) -> full[128*(4c + 2*half + u) + p, 512n + f]
    full = acc.transpose(0, 1, 3, 2, 4, 5).reshape(S, D)
    full = (full.astype(np.float64) + np.asarray(bo, np.float64)).astype(np.float32)
    if _run_kwargs is not None:
        _NC_CACHE["last_results"] = res
    return full.reshape(1, S, D)


# revision 20
# speedup vs baseline: 1.0149x; 1.0149x over previous
"""Causal multi-head attention (B=1, S=4096, D=1024, H=16, HD=64) on 8 TRN2
NeuronCores.

Sharding: tensor-parallel over heads. Core c owns heads [2c, 2c+1]: Wq/Wk/Wv
column slices (128 cols) and Wo row slice (128 rows). Each core computes its
partial output projection over the full sequence (bf16 partials, contiguous
unit-major layout); the host sums the 8 partials and adds bo.

Device kernel (per core; bf16 matmul operands, fp32 PSUM):
  - x fed pre-transposed (xT [D, S]); QT/KT = W.T @ xT, no on-chip transpose.
  - scores computed TRANSPOSED per 128-row sk tile: scT[sk, sq] = K_h @ Q_h^T
    (the two heads' 64-contraction matmuls run concurrently via PE row
    groups); 1/sqrt(HD) folded into Wq/bq on the host.
  - softmax: table EXP on ACT into bf16; causal narrowing (diagonal sk tiles
    only compute/exp/accumulate sq in [128t, 512)); one DVE triu-mask
    multiply on the 128-wide diagonal band.
  - a fraction of the exp tiles (late chunks, every 5th non-diagonal tile)
    runs on the DVE instead, as a Schraudolph affine into int16 bf16-bit
    patterns (bitcast back to bf16 for attn@V) to relieve the ACT engine.
  - denominators from a ones-column appended to V (row 64 of the attn@V
    accumulator); copy+reciprocal on DVE, partition-broadcast (gpsimd),
    rescale attn-out^T before the output projection.
  - all heavy PE work that is off the attention critical path (QKV
    projections for chunk c+1, V transposes, output projection for chunk
    c-1) is split into small units and drained as PE fillers at the TOP of
    each attention iteration, so the in-order PE queue never blocks behind
    a semaphore-waiting scores matmul and the PE stays continuously busy.
  - last chunk's rescale + output projection run in 128-column pieces to
    pipeline the tail (proj copies there go on the then-idle ACT).
"""
import sys

sys.path.insert(0, "/opt/trn_rl_repo")

import numpy as np

import concourse.bacc as bacc
import concourse.mybir as mybir
import concourse.tile as tile
from concourse.bass_utils import run_bass_kernel_spmd
from concourse.dve_ops import RECIPROCAL_APPROX_FAST, RECIP_APPROX_FAST_CONSTS

F32 = mybir.dt.float32
BF16 = mybir.dt.bfloat16
I16 = mybir.dt.int16
COPYF = mybir.ActivationFunctionType.Copy
EXPF = mybir.ActivationFunctionType.Exp

# Schraudolph bf16-bits exp for the DVE-offloaded tiles:
# int16 bits = round(x * SCH_A + SCH_B) ~= bf16(exp(x))
SCH_A = float(128.0 / np.log(2.0))
SCH_B = 127.0 * 128.0 - 5.5

S, D, H, HD = 4096, 1024, 16, 64
NCORES = 8
CPC = D // NCORES  # 128 head-dim columns per core (2 heads)
NS = S // 512      # 8 chunks of 512 along the sequence
ND = D // 128      # 8 contraction chunks for the projections


def _build_nc():
    nc = bacc.Bacc("TRN2", target_bir_lowering=False, debug=False,
                   num_devices=NCORES)
    xT = nc.dram_tensor("xT", [D, S], BF16, kind="ExternalInput").ap()
    w3 = nc.dram_tensor("w3", [128, 3, ND, 128], BF16, kind="ExternalInput").ap()
    wo = nc.dram_tensor("wo", [128, D], BF16, kind="ExternalInput").ap()
    b3 = nc.dram_tensor("b3", [CPC, 3], F32, kind="ExternalInput").ap()
    tri = nc.dram_tensor("tri", [128, 2, 128], BF16, kind="ExternalInput").ap()
    # batched unit-major output: out[c, half, p, u, n, f] =
    #   partial[128*(4c + 2*half + u) + p, 512n + f]
    out = nc.dram_tensor("out", [NS, 2, 128, 2, 2, 512], BF16,
                         kind="ExternalOutput").ap()

    with tile.TileContext(nc) as tc:
        _emit(nc, tc, xT, w3, wo, b3, tri, out)
    nc.compile()
    return nc


def _emit(nc, tc, xT, w3, wo, b3, tri, out):
    from collections import deque
    from contextlib import ExitStack
    ctx = ExitStack()
    with ctx:
        consts = ctx.enter_context(tc.tile_pool(name="consts", bufs=1))
        persist = ctx.enter_context(tc.tile_pool(name="persist", bufs=1))
        xt_pool = ctx.enter_context(tc.tile_pool(name="xt", bufs=3))
        vt_pool = ctx.enter_context(tc.tile_pool(name="vt", bufs=2))
        pt_pool = ctx.enter_context(tc.tile_pool(name="pt", bufs=8))
        pti_pool = ctx.enter_context(tc.tile_pool(name="pti", bufs=4))
        rec_pool = ctx.enter_context(tc.tile_pool(name="rec", bufs=8))
        bc_pool = ctx.enter_context(tc.tile_pool(name="bc", bufs=4))
        ost_pool = ctx.enter_context(tc.tile_pool(name="ost", bufs=3))
        ps_mm = ctx.enter_context(tc.tile_pool(name="psmm", bufs=2, space="PSUM"))
        ps_sc = ctx.enter_context(tc.tile_pool(name="pssc", bufs=2, space="PSUM"))
        ps_ot = ctx.enter_context(tc.tile_pool(name="psot", bufs=2, space="PSUM"))

        # ---- constants (wq+wk slices first: the preamble needs them) -----
        w_sb = consts.tile([128, 3, ND, 128], BF16)
        nc.sync.dma_start(out=w_sb[:, 0:2], in_=w3[:, 0:2])
        b3_sb = consts.tile([CPC, 3], F32)
        bq_sb, bk_sb, bv_sb = b3_sb[:, 0:1], b3_sb[:, 1:2], b3_sb[:, 2:3]
        ti_sb = consts.tile([128, 2, 128], BF16)
        tri_sb, id_sb = ti_sb[:, 0], ti_sb[:, 1]
        wo_sb = consts.tile([128, D], BF16)

        def emit_small_consts():
            nc.sync.dma_start(out=b3_sb, in_=b3)
            nc.sync.dma_start(out=ti_sb, in_=tri)

        # ---- persistent activations -----------------------------------
        QT = persist.tile([128, S], BF16)     # [c(2 heads x 64), s]
        KT = persist.tile([128, S], BF16)
        VP = persist.tile([128, 2, 32, 65], BF16)  # V natural + ones col
        OT = persist.tile([128, S], BF16)     # rescaled attn out^T, 2 heads

        xT_k = xT.rearrange("(k p) s -> p k s", p=128)

        def load_xt(c):
            xt = xt_pool.tile([128, ND, 512], BF16, tag="xt")
            cc0 = slice(512 * c, 512 * (c + 1))
            nc.sync.dma_start(out=xt[:, 0:4], in_=xT_k[:, 0:4, cc0])
            nc.sync.dma_start(out=xt[:, 4:ND], in_=xT_k[:, 4:ND, cc0])
            return [xt[:, k] for k in range(ND)]

        # ---- filler machinery -----------------------------------------
        qkv_q = deque()   # QKV/transpose units (deadline: end of prev loop)
        proj_q = deque()  # output-projection units (soft deadline)

        def mk_proj_unit(i, half, st, cc, which):
            # one half (4 accumulation steps) of projection `which` in
            # (q, k, v); the second half adds the bias
            def u():
                if half == 0:
                    st[i] = ps_mm.tile([128, 512], F32, tag="mm",
                                       name=f"pqkv{i}")
                p_ps = st[i]
                for k in range(4 * half, 4 * half + 4):
                    nc.tensor.matmul(p_ps, w_sb[:, i, k], st["xts"][k],
                                     start=(k == 0), stop=(k == ND - 1))
                if half == 1:
                    if i == 0:
                        nc.vector.tensor_scalar_add(QT[:, cc], p_ps, bq_sb)
                    elif i == 1:
                        nc.vector.tensor_scalar_add(KT[:, cc], p_ps, bk_sb)
                    else:
                        st["vt"] = vt_pool.tile([128, 512], BF16, tag="vt",
                                                name="vt")
                        nc.vector.tensor_scalar_add(st["vt"], p_ps, bv_sb)
            return u

        def mk_tr_unit(c, t, st):
            def u():
                j = 4 * c + t
                tr_ps = ps_mm.tile([128, 128], BF16, tag="mm", name="trps")
                nc.tensor.transpose(tr_ps, st["vt"][:, 128 * t:128 * (t + 1)],
                                    id_sb)
                nc.vector.tensor_copy(VP[:, 0, j, 0:64], tr_ps[:, 0:64])
                nc.vector.tensor_copy(VP[:, 1, j, 0:64], tr_ps[:, 64:128])
                nc.vector.memset(VP[:, 0, j, 64:65], 1.0)
                nc.vector.memset(VP[:, 1, j, 64:65], 1.0)
            return u

        def enqueue_qkv(c, xts, skip_qk=False):
            cc = slice(512 * c, 512 * (c + 1))
            st = {"xts": xts}
            for i in range(3):
                if skip_qk and i < 2:
                    continue
                for half in (0, 1):
                    qkv_q.append(mk_proj_unit(i, half, st, cc, i))
            for t in range(4):
                qkv_q.append(mk_tr_unit(c, t, st))
            return st

        ogrp = {}  # (c, half) -> [staging tile, units done]

        def emit_out_unit(c, t, n, copy_on_act=False, split_dma=False):
            key = (c, t // 2)
            if key not in ogrp:
                ogrp[key] = [ost_pool.tile([128, 2, 2, 512], BF16, tag="ost",
                                           name="obig"), 0]
            o_big = ogrp[key][0]
            ss = slice(128 * (4 * c + t), 128 * (4 * c + t + 1))
            nn = slice(512 * n, 512 * (n + 1))
            pr_ps = ps_mm.tile([128, 512], F32, tag="mm", name="prps")
            nc.tensor.matmul(pr_ps, OT[:, ss], wo_sb[:, nn],
                             start=True, stop=True)
            if copy_on_act:
                nc.scalar.activation(out=o_big[:, t % 2, n], in_=pr_ps,
                                     func=COPYF)
            else:
                nc.vector.tensor_copy(o_big[:, t % 2, n], pr_ps)
            ogrp[key][1] += 1
            if split_dma and ogrp[key][1] % 2 == 0:
                nc.sync.dma_start(out=out[c, t // 2][:, t % 2],
                                  in_=o_big[:, t % 2])
                if ogrp[key][1] == 4:
                    del ogrp[key]
            elif not split_dma and ogrp[key][1] == 4:
                nc.sync.dma_start(out=out[c, t // 2], in_=o_big)
                del ogrp[key]

        def drain_one():
            if qkv_q:
                qkv_q.popleft()()
                return True
            if proj_q:
                emit_out_unit(*proj_q.popleft())
                return True
            return False

        # ---- preamble --------------------------------------------------
        # PE p-state warmup on zeros while the first DMAs land
        warm = consts.tile([128, 512], BF16)
        nc.vector.memset(warm, 0.0)
        wps = ps_mm.tile([128, 512], F32, tag="mm", name="wps")
        for _ in range(8):
            nc.tensor.matmul(wps, warm[:, 0:128], warm, start=True, stop=True)
        xts0 = load_xt(0)
        emit_small_consts()
        xts1 = load_xt(1)
        nc.sync.dma_start(out=w_sb[:, 2], in_=w3[:, 2])
        nc.sync.dma_start(out=wo_sb, in_=wo)
        st0 = {"xts": xts0}
        for i in (0, 1):          # chunk-0 Q and K projections, inline
            for half in (0, 1):
                mk_proj_unit(i, half, st0, slice(0, 512), i)()
        # chunk-0 V + transposes and chunk-1 QKV go through the filler queue
        st0b = enqueue_qkv(0, xts0, skip_qk=True)

        xts_next = xts1
        for c in range(NS):
            cc = slice(512 * c, 512 * (c + 1))
            if 0 < c < NS - 1:
                enqueue_qkv(c + 1, xts_next)
                xts_next = load_xt(c + 2) if c + 2 < NS else None

            njt = 4 * (c + 1)
            ot0 = ps_ot.tile([65, 512], F32, tag="ot", name="ot0")
            ot1 = ps_ot.tile([65, 512], F32, tag="ot", name="ot1")

            def emit_av(j, lo, pt, njt=njt, ot0=ot0, ot1=ot1):
                st, sp = (j == 0), (j == njt - 1)
                nc.tensor.matmul(ot0[:, lo:512], VP[:, 0, j], pt[:, 0, lo:512],
                                 start=st, stop=sp, skip_group_check=True)
                nc.tensor.matmul(ot1[:, lo:512], VP[:, 1, j], pt[:, 1, lo:512],
                                 start=st, stop=sp, skip_group_check=True)

            # how many fillers to drain per iteration to empty qkv_q in time
            pending = []
            for j in range(njt):
                need = len(qkv_q) + max(0, len(proj_q) - 4)
                rate = max(1, -(-need // max(1, njt - j)))
                for _ in range(min(rate, 3 if njt <= 8 else 8)):
                    drain_one()
                jj = slice(128 * j, 128 * (j + 1))
                t = j - 4 * c
                lo = 128 * t if t >= 0 else 0
                cl = slice(512 * c + lo, 512 * (c + 1))
                sc = ps_sc.tile([128, 2, 512], F32, tag="sc", name="sc")
                nc.tensor.matmul(sc[:, 0, lo:512], KT[0:64, jj], QT[0:64, cl],
                                 start=True, stop=True)
                nc.tensor.matmul(sc[:, 1, lo:512], KT[64:128, jj],
                                 QT[64:128, cl], start=True, stop=True)
                if c >= 3 and t < 0 and j % 5 == 2:
                    # offload this tile's exp to the DVE (Schraudolph
                    # affine into bf16 bit patterns)
                    pti = pti_pool.tile([128, 2, 512], I16, tag="pti",
                                        name="pti")
                    nc.vector.tensor_scalar(pti, sc[:, :, :],
                                            SCH_A, SCH_B,
                                            mybir.AluOpType.mult,
                                            mybir.AluOpType.add)
                    pt = pti.bitcast(BF16)
                else:
                    pt = pt_pool.tile([128, 2, 512], BF16, tag="pt", name="pt")
                    nc.scalar.activation(out=pt[:, :, lo:512],
                                         in_=sc[:, :, lo:512], func=EXPF)
                    if t >= 0:  # diagonal 128-wide band: triu mask
                        for h in (0, 1):
                            nc.vector.tensor_mul(pt[:, h, lo:lo + 128],
                                                 pt[:, h, lo:lo + 128], tri_sb)
                pending.append((j, lo, pt))
                if len(pending) > 2:
                    emit_av(*pending.pop(0))
            # flush: interleave fillers between the exp-blocked tail avs
            for p in pending:
                drain_one()
                if len(qkv_q) + len(proj_q) >= 6:
                    drain_one()
                emit_av(*p)

            if c < NS - 1:
                # softmax denominators (row 64) -> rescale attn-out^T
                bcs = []
                for h, ot in ((0, ot0), (1, ot1)):
                    den = rec_pool.tile([1, 512], F32, tag="den", name="den")
                    nc.vector.tensor_copy(den, ot[64:65, :])
                    rec = rec_pool.tile([1, 512], F32, tag="rec", name="rec")
                    nc.vector._custom_dve(RECIPROCAL_APPROX_FAST, out=rec,
                                          in0=den,
                                          s0=RECIP_APPROX_FAST_CONSTS["s0"],
                                          s1=RECIP_APPROX_FAST_CONSTS["s1"],
                                          imm2=RECIP_APPROX_FAST_CONSTS["imm2"])
                    bc = bc_pool.tile([64, 512], F32, tag="bc", name="bc")
                    nc.gpsimd.partition_broadcast(bc, rec)
                    bcs.append(bc)
                for h, ot in ((0, ot0), (1, ot1)):
                    nc.vector.tensor_mul(OT[64 * h:64 * (h + 1), cc],
                                         ot[0:64, :], bcs[h])
                if c == 0:
                    enqueue_qkv(1, xts_next)
                    xts_next = load_xt(2)
                while qkv_q:       # chunk c+1's QKV must be done before its loop
                    drain_one()
                proj_q.extend((c, t, n, False) for t in range(4)
                              for n in range(2))
            else:
                # tail: 128-col pieces pipelined (rescale_t || proj_{t-1});
                # copies on the now-idle ACT
                while qkv_q:
                    drain_one()
                for t in range(4):
                    pp = slice(128 * t, 128 * (t + 1))
                    ccp = slice(512 * c + 128 * t, 512 * c + 128 * (t + 1))
                    for h, ot in ((0, ot0), (1, ot1)):
                        den = rec_pool.tile([1, 128], F32, tag="den",
                                            name="den")
                        nc.vector.tensor_copy(den, ot[64:65, pp])
                        rec = rec_pool.tile([1, 128], F32, tag="rec",
                                            name="rec")
                        nc.vector._custom_dve(
                            RECIPROCAL_APPROX_FAST, out=rec, in0=den,
                            s0=RECIP_APPROX_FAST_CONSTS["s0"],
                            s1=RECIP_APPROX_FAST_CONSTS["s1"],
                            imm2=RECIP_APPROX_FAST_CONSTS["imm2"])
                        bc = bc_pool.tile([64, 128], F32, tag="bc", name="bc")
                        nc.gpsimd.partition_broadcast(bc, rec)
                        nc.vector.tensor_mul(OT[64 * h:64 * (h + 1), ccp],
                                             ot[0:64, pp], bc)
                    if proj_q:
                        emit_out_unit(*proj_q.popleft())
                    if t > 0:
                        emit_out_unit(c, t - 1, 0, True, split_dma=True)
                        emit_out_unit(c, t - 1, 1, True, split_dma=True)
                emit_out_unit(c, 3, 0, True, split_dma=True)
                emit_out_unit(c, 3, 1, True, split_dma=True)

        while proj_q:
            emit_out_unit(*proj_q.popleft())


_NC_CACHE = {}


def _get_nc():
    if "nc" not in _NC_CACHE:
        _NC_CACHE["nc"] = _build_nc()
    return _NC_CACHE["nc"]


def make_in_maps(x, Wq, bq, Wk, bk, Wv, bv, Wo, bo):
    import ml_dtypes
    cdt = ml_dtypes.bfloat16
    x = np.asarray(x, np.float32).reshape(S, D)
    xT = np.ascontiguousarray(x.T).astype(cdt)
    scale = 1.0 / np.sqrt(HD)
    tri = np.stack([np.triu(np.ones((128, 128), np.float32)),
                    np.eye(128, dtype=np.float32)], axis=1)
    in_maps = []
    for c in range(NCORES):
        cs = slice(CPC * c, CPC * (c + 1))
        # w3[p, proj, k, c2] = W[128k+p, c2] for the three projections
        w3 = np.stack([np.asarray(Wq)[:, cs] * scale,
                       np.asarray(Wk)[:, cs],
                       np.asarray(Wv)[:, cs]], axis=1)  # [D, 3, 128]
        w3 = np.ascontiguousarray(
            w3.reshape(ND, 128, 3, CPC).transpose(1, 2, 0, 3)).astype(cdt)
        in_maps.append({
            "xT": xT,
            "w3": w3,
            "wo": np.ascontiguousarray(np.asarray(Wo)[cs, :]).astype(cdt),
            "b3": np.stack([np.asarray(bq)[cs] * scale,
                            np.asarray(bk)[cs],
                            np.asarray(bv)[cs]], axis=1).astype(np.float32),
            "tri": tri.astype(cdt),
        })
    return in_maps


def kernel(x, Wq, bq, Wk, bk, Wv, bv, Wo, bo, _run_kwargs=None):
    nc = _get_nc()
    in_maps = make_in_maps(x, Wq, bq, Wk, bk, Wv, bv, Wo, bo)
    res = run_bass_kernel_spmd(nc, in_maps, list(range(NCORES)),
                               **(_run_kwargs or {}))
    acc = np.zeros((NS, 2, 128, 2, 2, 512), np.float32)
    for c in range(NCORES):
        acc += res.results[c]["out"].astype(np.float32)
    # (c, half, p, u, n, f) -> full[128*(4c + 2*half + u) + p, 512n + f]
    full = acc.transpose(0, 1, 3, 2, 4, 5).reshape(S, D)
    full = (full.astype(np.float64) + np.asarray(bo, np.float64)).astype(np.float32)
    if _run_kwargs is not None:
        _NC_CACHE["last_results"] = res
    return full.reshape(1, S, D)
